# revision 98
# baseline (speedup 1.0000x reference)
"""Trainium2 Bass kernel for the AttentionBlock problem.

Full inputs:  x [16, 64, 64, 64] f32, w_theta [8, 64], w_phi [8, 64],
              w_g [32, 64], w_o [64, 32], gamma [] (all f32).
Sharding: data-parallel over batch, 2 samples per core on 8 NeuronCores.

Per-sample math (C=64, S=4096, T=S/4=1024):
  theta = w_theta @ x            [8, S]
  phi   = pool2x2(w_phi @ x)     [8, T]
  g     = pool2x2(w_g @ x)       [32, T]
  scoresT[t, s] = sum_c phi[c, t] theta[c, s]
  expT = exp(scoresT)            (no max-subtraction; |scores| <~ 20 is fp32-safe)
  gw[t, m] = sum_c g[c, t] wog[c, m]   (wog = (gamma*w_o)^T, folded on host)
  Fused attn + output conv in ONE accumulation per (chunk, half):
    lhsT = gwo[t-tile] = [gw (64 cols) | ones (64 cols)]  (bf16 [128, 128])
    psum rows 0:64  = oU[m, s] = sum_t gw[t, m] expT[t, s] = gamma*(w_o@attnU)
    psum rows 64:128 = Z[s] broadcast across 64 partitions
  out = oU * recip(Z) + x        (DVE reciprocal + mul + residual add)

This formulation has no separate output conv, no g transposes and no
attnS intermediate; the 2x2 maxpool of phi+g is ONE tensor_reduce
(axis=XY) straight from the conv's PSUM; only theta is staged to SBUF.

Schedule (per core): sample-0 conv/pool/gw phase (wct packed into the
head of x so one DMA delivers both), then the 8 (sample, chunk) bodies
run a two-stage pipeline over 8 t-tile slots:
  body n, slot t: scores+exp(chunk n, tile t); attn of chunk n-1
  interleaved (h0 over slots 0..2, h1 over 3..5) so each [128,512] PSUM
  accumulator frees mid-body, 2 banks suffice, and both norm chains
  clear well before the body ends; recip+mul+add+store of each half is
  emitted right after its accumulation stops. The next sample's
  conv/pool/gw units are spread 3-per-body at slots 3/5/7 so their
  PE/DVE bursts never delay the next body's score tiles.
Each body's t0 scores are pre-emitted at the end of the previous body
(and the last body's t7 scores at slot 6) so no exp ever waits on a
score matmul stuck behind boundary work in the in-order PE queue.
The last chunk additionally interleaves its OWN attn per slot (PSUM from
the then-idle conv pool), splits its final exp in half, and runs one
tail reciprocal on ACT (table load overlaps the DVE chain), so the tail
after the final exp is ~2 matmuls + two overlapped norm chains + store.
ACT (exp, ~67us busy) is the critical engine; PE (~60us) and DVE (~45us)
hide under it. Measured: 80746 ns (baseline 86019 ns).
"""

import sys

if "/opt/trn_rl_repo" not in sys.path:
    sys.path.insert(0, "/opt/trn_rl_repo")

import ml_dtypes
import numpy as np

import concourse.bass as bass
import concourse.tile as tile
from concourse import bacc, mybir
from concourse.bass_utils import run_bass_kernel_spmd

F32 = mybir.dt.float32
F32R = mybir.dt.float32r
BF16 = mybir.dt.bfloat16
AF = mybir.ActivationFunctionType
ALU = mybir.AluOpType

B, C, H, W = 16, 64, 64, 64
S = H * W            # 4096
T = S // 4           # 1024
NCORES = 8
BLOC = B // NCORES   # 2 samples per core
NT = T // 128        # 8 t-tiles
CHUNK = 1024         # s-chunk size
NCH = S // CHUNK     # 4 chunks per sample

_OUT = [None]
_WOGEXT = [None]
_WCTSB = [None]
_COPY1_HOOK = [None]
_HEAD = [None]

# prev-chunk attn schedule: slot t -> (half, [t-tiles]); both halves finish
# by slot 5 so their norms (DVE) clear well before the body ends and the
# last body's tail only carries its own two half-norms
_PREV_SCHED = {0: (0, [0, 1, 2]), 1: (0, [3, 4, 5]), 2: (0, [6, 7]),
               3: (1, [0, 1, 2]), 4: (1, [3, 4, 5]), 5: (1, [6, 7])}
_PREV_NORM = {2: 0, 5: 1}
# fine (last-chunk own attn) schedule: slot t -> [(half, t-tile)]; a tile
# only appears at slot >= tile+1 so PE never waits on ACT, and h0/h1 first
# appear at slots 1/2 (their PSUM comes from the conv pool, idle by then)
_FINE_SCHED = {1: [(0, 0)], 2: [(0, 1), (1, 0)], 3: [(0, 2), (1, 1)],
               4: [(0, 3), (1, 2)], 5: [(0, 4), (1, 3)],
               6: [(0, 5), (1, 4)], 7: [(0, 6), (1, 5), (1, 6)]}
_FINE_POST = [(0, 7), (1, 7)]


def _phase_a(nc, tc, pools, s, x_ext, wog_sb, gwinit_ext):
    """Load x, fused convs, merged maxpools, gw matmul. Returns handles."""
    (pp_sc, pp_at, pp_sm, p_samp, p_chunk) = pools

    # ---- load x: [64, 4096] f32 on partitions 0:64 (base 0 everywhere so
    # SBUF+SBUF tensor ops against it satisfy the equal-base-partition rule)
    # xw_sb: [64, 128 wct | 4096 x] — wct rides in the head of sample 0's
    # x tensor so ONE DMA (one 625ns HWDGE issue slot) delivers both and
    # every later x chunk moves up a slot. x slices live at cols 128+.
    xw_sb = p_samp.tile([64, 128 + S], F32R, tag="x_sb")
    x_sb = xw_sb[:, 128:]
    if s == 0:
        qs = [(0, 640)] + [(640 + q * 512, 640 + (q + 1) * 512)
                           for q in range(7)]
    else:
        qs = [(128, 2176), (2176, 4224)]
    for lo, hi in qs:
        nc.sync.dma_start(xw_sb[:, lo:hi], x_ext[s, :, lo:hi])
    if s == 0:
        # needed only by the gw matmuls ~15us in; keep it out of the
        # latency-critical head of the queue
        nc.sync.dma_start(wog_sb[32:64, :], _WOGEXT[0][:])

    # ---- fused 1x1 convs: [128, 512] psum chunks; conv output rows 0:8
    # theta, 64:72 phi(unpooled), 96:128 g(unpooled): phi+g live in the
    # aligned 64:128 partition window so one op pools both (the DVE/Pool
    # BIR rule forbids a 64-partition access starting at partition 32).
    # Only theta is staged to SBUF (tpg_sb); phi/g are pooled from PSUM.
    tpg_sb = p_samp.tile([8, 4096], F32R, tag="tpg_sb")
    # pooled phi+g: one merged buffer; phi lands on rows 0:8, g on 32:64
    pg_sb = p_samp.tile([64, T], F32R, tag="pg_sb")
    # gwo: per t-tile [128, 128] = [gw 64 cols | ones 64 cols] bf16
    gwo_sb = p_samp.tile([128, NT * 128], BF16, tag="gwo_sb")

    def pool2x2(dst, src):
        # full 2x2 maxpool of one conv chunk in ONE tensor_reduce: src
        # [p, 512] viewed [p, q(h-pair), hh, wo, ww] with the two pooled
        # axes innermost, reduced via axis=XY. Single input, so reading
        # straight from PSUM is legal (one-PSUM-operand rule).
        sv = src.rearrange("p (q hh wo ww) -> p q wo hh ww",
                           q=4, hh=2, wo=32, ww=2)
        dv = dst.rearrange("p (q wo) -> p q wo", wo=32)
        nc.vector.tensor_reduce(dv, sv, mybir.AxisListType.XY, ALU.max)

    # conv chunk k covers h rows 8k..8k+8 == phi/g t-tile k; pool per chunk
    # so downstream scores can start before the whole sample is done.
    # pool2x2 reads the PSUM conv output directly; only theta is staged.
    if s == 0:
        _WCTSB[0] = xw_sb[:, 0:128]

    def conv_chunk(k):
        ps_conv = pp_sm.tile([128, 512], F32, tag="sm", name=f"ps_conv_{s}_{k}")
        nc.tensor.matmul(
            ps_conv[:],
            _WCTSB[0],
            x_sb[:, k * 512:(k + 1) * 512],
            start=True, stop=True,
        )
        # pool phi (rows 64:72) and g (rows 96:128) in one op over rows 64:128
        pool2x2(pg_sb[:, k * 128:(k + 1) * 128],
                ps_conv[64:128, :].bitcast(F32R))
        if s == 0 and k < 2:
            # ACT is idle during startup; the first chunks' theta is the
            # latency-critical piece. copy1 is deferred into the exp-t0
            # split (between the two halves) so exp-t0a doesn't sit behind
            # it in the in-order ACT queue.
            def cp(k=k):
                nc.scalar.copy(tpg_sb[:, k * 512:(k + 1) * 512],
                               ps_conv[0:8, :])
            if k == 1:
                _COPY1_HOOK[0] = cp
            else:
                cp()
        else:
            nc.vector.tensor_copy(tpg_sb[:, k * 512:(k + 1) * 512],
                                  ps_conv[0:8, :])

    def gw_unit():
        # gwinit (ones columns) queued late so it never delays theta/x DMAs.
        nc.sync.dma_start(gwo_sb[:], gwinit_ext[:])
        # gw tiles: gw[t, m] = sum_c g[c, t] wog[c, m]; lhsT = g block
        # [32, 128] (partitions 32:64 of pg_sb), rhs = wog held at
        # partitions 32:64 so operand partition bases match
        for t in range(NT):
            ps_gw = pp_sm.tile([128, 64], F32, tag="sm", name=f"ps_gw_{s}_{t}")
            nc.tensor.matmul(
                ps_gw[:], pg_sb[32:64, t * 128:(t + 1) * 128],
                wog_sb[32:64, :], start=True, stop=True,
            )
            nc.vector.tensor_copy(gwo_sb[:, t * 128:t * 128 + 64], ps_gw[:])

    handles = (x_sb, tpg_sb, pg_sb, gwo_sb)
    units = [lambda k=k: conv_chunk(k) for k in range(8)] + [gw_unit]
    return handles, units


def _act_reciprocal(nc, out, in_):
    """Reciprocal on the ACT engine (table-based, ~1e-3 accurate — fine
    for the 2e-2 gate; bass's wrapper refuses it, so emit the raw
    instruction). Used only in the tail where ACT is idle and the DVE
    norm chain is the critical path."""
    inputs = [nc.scalar.lower_ap(in_)]
    for v in (0.0, 1.0, 0.0):  # bias, scale, alpha immediates
        inputs.append(mybir.ImmediateValue(dtype=mybir.dt.float32, value=v))
    return nc.scalar.add_instruction(
        mybir.InstActivation(
            name=nc.scalar.bass.get_next_instruction_name(),
            func=AF.Reciprocal,
            ins=inputs,
            outs=[nc.scalar.lower_ap(out)],
        )
    )


def _emit_half(nc, pools, s, ch, h, hs, ps_at, act_recip=False,
               dma_engine=None):
    """norm (oU * 1/Z) + residual add + store for one 512-half.
    reciprocal_approx_fast is ~5x cheaper than reciprocal() and accurate
    to ~51 ULP; Z >= 1 always (sum of 1024 positive exps) so its edge
    cases are unreachable. DVE divide doesn't exist at ISA level."""
    (pp_sc, pp_at, pp_sm, p_samp, p_chunk) = pools
    x_sb = hs[0]
    rz_sb = p_chunk.tile([64, 512], F32, tag="rz_sb",
                         name=f"rz_{s}_{ch}_{h}", bufs=2)
    out_sb = p_chunk.tile([64, 512], F32, tag="out_sb",
                          name=f"out_{s}_{ch}_{h}", bufs=3)
    if act_recip:
        _act_reciprocal(nc, rz_sb[:], ps_at[64:128, :])
    else:
        nc.vector.reciprocal(rz_sb[:], ps_at[64:128, :])
    nc.vector.tensor_mul(out_sb[:], ps_at[0:64, :], rz_sb[:])
    s0 = ch * CHUNK + h * 512
    nc.vector.tensor_add(
        out_sb[:], out_sb[:],
        x_sb[:, s0:s0 + 512].bitcast(F32),
    )
    (dma_engine or nc.sync).dma_start(_OUT[0][s, :, s0:s0 + 512], out_sb[:])


def _emit_chunk(nc, pools, s, ch, handles, prev, fine=False, units=(),
                nxt=None):
    """Pipeline body: scores+exp for chunk (s, ch) over 8 t-tile slots,
    with prev chunk's fused attn+oconv matmuls interleaved per
    _PREV_SCHED and (if fine) this chunk's own attn per _FINE_SCHED.
    Deferred phase-A units (next sample's convs/gw) are emitted at mid-
    body slots so they never delay the boundary score tiles. The NEXT
    body's t0 scores are emitted at the end of this body (and the fine
    body's t7 scores at slot 6) so no exp ever waits on a score matmul
    that sits behind boundary work in the in-order PE queue."""
    (pp_sc, pp_at, pp_sm, p_samp, p_chunk) = pools
    units = list(units)

    at_prev = [None, None]
    if prev is not None:
        ps_, ch_, expT_ = prev
        gwo_ = handles[ps_][3]
        for h in range(2):
            at_prev[h] = pp_at.tile([128, 512], F32, tag="at",
                                    name=f"ps_at_{ps_}_{ch_}_{h}")

    x_sb, tpg_sb, pg_sb, gwo_sb = handles[s]
    theta = tpg_sb[:]
    expT = p_chunk.tile([128, NT * CHUNK], BF16, tag="expT",
                        name=f"expT_{s}_{ch}", bufs=3)
    at_cur = [None, None]

    def fine_mm(h, tt):
        if at_cur[h] is None:
            at_cur[h] = pp_sm.tile([128, 512], F32, tag="sm",
                                   name=f"ps_at_{s}_{ch}_{h}")
        nc.tensor.matmul(
            at_cur[h][:],
            gwo_sb[:, tt * 128:(tt + 1) * 128],
            expT[:, tt * CHUNK + h * 512:tt * CHUNK + (h + 1) * 512],
            start=(tt == 0), stop=(tt == NT - 1),
        )

    def sc_mm(ps_sc, t, hh):
        nc.tensor.matmul(
            ps_sc[:, hh * 512:(hh + 1) * 512],
            pg_sb[0:8, t * 128:(t + 1) * 128],
            theta[:, ch * CHUNK + hh * 512:ch * CHUNK + (hh + 1) * 512],
            start=True, stop=True,
        )

    pre_t0, _HEAD[0] = _HEAD[0], None
    pre_t7 = None
    for t in range(NT):
        if t == 0 and pre_t0 is not None:
            ps_sc = pre_t0          # scores already emitted last body
            mms_done = True
        elif t == NT - 1 and pre_t7 is not None:
            ps_sc = pre_t7          # scores already emitted at slot 6
            mms_done = True
        elif s == 0 and ch == 0 and t == 0:
            ps_sc = None            # per-half tiles (see split below)
            mms_done = False
        else:
            ps_sc = pp_sc.tile([128, CHUNK], F32, tag="sc",
                               name=f"ps_sc_{s}_{ch}_{t}")
            mms_done = False
        if (s == 0 and ch == 0 and t == 0) or (fine and t == NT - 1):
            # split boundary tiles in half: the first so ACT starts as soon
            # as conv0's theta half is staged (theta's copy1 is emitted
            # between the halves, keeping exp-t0a ahead of it in the
            # in-order ACT queue; separate per-half tiles also shift the
            # slot rotation a step earlier for the whole stream); the last
            # so the tail's h0 attn+norm chain fires after only the first
            # 512 columns
            for hh in range(2):
                if ps_sc is None:
                    ph = pp_sc.tile([128, 512], F32, tag="sc",
                                    name=f"ps_sc_{s}_{ch}_0{'ab'[hh]}")
                    nc.tensor.matmul(
                        ph[:], pg_sb[0:8, 0:128],
                        theta[:, hh * 512:(hh + 1) * 512],
                        start=True, stop=True,
                    )
                    src = ph[:]
                else:
                    if not mms_done:
                        sc_mm(ps_sc, t, hh)
                    src = ps_sc[:, hh * 512:(hh + 1) * 512]
                nc.scalar.activation(
                    expT[:, t * CHUNK + hh * 512:t * CHUNK + (hh + 1) * 512],
                    src, AF.Exp,
                )
                if hh == 0 and _COPY1_HOOK[0] is not None:
                    _COPY1_HOOK[0]()
                    _COPY1_HOOK[0] = None
        else:
            if not mms_done:
                for hh in range(CHUNK // 512):
                    sc_mm(ps_sc, t, hh)
            nc.scalar.activation(
                expT[:, t * CHUNK:(t + 1) * CHUNK], ps_sc[:], AF.Exp
            )
        if fine and t == NT - 3:
            # pre-emit the final tile's scores (two slots early) so its
            # exps never wait on PE reaching them behind late-body work
            pre_t7 = pp_sc.tile([128, CHUNK], F32, tag="sc",
                                name=f"ps_sc_{s}_{ch}_{NT - 1}")
            sc_mm(pre_t7, NT - 1, 0)
            sc_mm(pre_t7, NT - 1, 1)
        if prev is not None and t in _PREV_SCHED:
            h, tts = _PREV_SCHED[t]
            for tt in tts:
                nc.tensor.matmul(
                    at_prev[h][:],
                    gwo_[:, tt * 128:(tt + 1) * 128],
                    expT_[:, tt * CHUNK + h * 512:tt * CHUNK + (h + 1) * 512],
                    start=(tt == 0), stop=(tt == NT - 1),
                )
            if t in _PREV_NORM:
                _emit_half(nc, pools, ps_, ch_, _PREV_NORM[t],
                           handles[ps_], at_prev[_PREV_NORM[t]])
        if fine:
            for h, tt in _FINE_SCHED.get(t, ()):
                fine_mm(h, tt)
        if t in (3, 5, 7) and units:
            units.pop(0)()

    if fine:
        for h, tt in _FINE_POST:
            fine_mm(h, tt)
        # tail norms: h0's recip on DVE (starts immediately), h1's on ACT
        # (its table load overlaps h0's DVE chain); the final store issues
        # from the idle ACT queue so the two out-DMAs' 625ns HWDGE issue
        # slots run in parallel
        _emit_half(nc, pools, s, ch, 0, handles[s], at_cur[0])
        _emit_half(nc, pools, s, ch, 1, handles[s], at_cur[1],
                   act_recip=True)
        return None
    if nxt is not None:
        # head of the next body: its t0 scores, emitted here so they run
        # ahead of the boundary and its first exp starts back-to-back
        s2, ch2 = nxt
        x2, tpg2, pg2, gwo2 = handles[s2]
        ps_head = pp_sc.tile([128, CHUNK], F32, tag="sc",
                             name=f"ps_sc_{s2}_{ch2}_0")
        for hh in range(2):
            nc.tensor.matmul(
                ps_head[:, hh * 512:(hh + 1) * 512],
                pg2[0:8, 0:128],
                tpg2[:, ch2 * CHUNK + hh * 512:ch2 * CHUNK + (hh + 1) * 512],
                start=True, stop=True,
            )
        _HEAD[0] = ps_head
    return (s, ch, expT)


def build_nc():
    nc = bacc.Bacc("TRN2", target_bir_lowering=False, debug=False,
                   num_devices=NCORES)
    x_ext = nc.dram_tensor("x", [BLOC, C, 128 + S], F32R,
                            kind="ExternalInput").ap()
    wog_ext = nc.dram_tensor("wog", [32, 64], F32R, kind="ExternalInput").ap()
    gwinit_ext = nc.dram_tensor("gwinit", [128, NT * 128], BF16,
                                kind="ExternalInput").ap()
    out_ext = nc.dram_tensor("out", [BLOC, C, S], F32, kind="ExternalOutput").ap()

    with tile.TileContext(nc) as tc:
        with (
            tc.tile_pool(name="wpool", bufs=1) as p_w,
            tc.tile_pool(name="samp", bufs=2) as p_samp,
            tc.tile_pool(name="chunk", bufs=2) as p_chunk,
            tc.tile_pool(name="ppsc", bufs=2, space="PSUM") as pp_sc,
            tc.tile_pool(name="ppat", bufs=2, space="PSUM") as pp_at,
            tc.tile_pool(name="ppsm", bufs=2, space="PSUM") as pp_sm,
        ):
            wog_sb = p_w.tile([64, 64], F32R, tag="wog_sb")
            # dummy exp with no data deps: hoists the ACT exp-table load
            # (1.3us) to t=0, off the conv0 -> copy -> first-exp chain
            dummy_sb = p_w.tile([1, 2], F32, tag="dummy_sb")
            nc.vector.memset(dummy_sb[:], 0)
            nc.scalar.activation(dummy_sb[:], dummy_sb[:], AF.Exp)
            _WOGEXT[0] = wog_ext
            _OUT[0] = out_ext
            pools = (pp_sc, pp_at, pp_sm, p_samp, p_chunk)
            handles = [None] * BLOC
            handles[0], units0 = _phase_a(nc, tc, pools, 0, x_ext,
                                          wog_sb, gwinit_ext)
            # sample 0's convs run inline (they feed body (0,0) directly);
            # its gw unit is deferred into body (0,0)'s mid-slots, giving
            # the otherwise attn-less first body PE work that keeps the
            # array's p-state hot for the boundary head matmuls
            for u in units0[:-1]:
                u()
            prev = None
            pending = [units0[-1]]  # deferred phase-A work units
            seq = [(s, ch) for s in range(BLOC) for ch in range(NCH)]
            for i, (s, ch) in enumerate(seq):
                last = i == len(seq) - 1
                nxt = None if last else seq[i + 1]
                prev = _emit_chunk(nc, pools, s, ch, handles, prev,
                                   fine=last, units=pending[:3], nxt=nxt)
                pending = pending[3:]
                if ch == 0 and s + 1 < BLOC:
                    handles[s + 1], pending = _phase_a(
                        nc, tc, pools, s + 1, x_ext, wog_sb,
                        gwinit_ext)

    nc.compile()
    return nc


_NC_CACHE = None


def _get_nc():
    global _NC_CACHE
    if _NC_CACHE is None:
        _NC_CACHE = build_nc()
    return _NC_CACHE


def kernel(x, w_theta, w_phi, w_g, w_o, gamma):
    x = np.ascontiguousarray(np.asarray(x, dtype=np.float32))
    w_theta = np.asarray(w_theta, dtype=np.float32)
    w_phi = np.asarray(w_phi, dtype=np.float32)
    w_g = np.asarray(w_g, dtype=np.float32)
    w_o = np.asarray(w_o, dtype=np.float32)
    gamma_f = float(np.asarray(gamma, dtype=np.float32))

    # lhsT for the fused conv: [64, 128] = [w_theta.T | pad | w_phi.T | pad |
    # w_g.T] (phi at col 64, g at col 96 so phi+g land in the aligned 64:128
    # partition window of the conv output).
    wct = np.zeros((64, 128), dtype=np.float32)
    wct[:, 0:8] = w_theta.T
    wct[:, 64:72] = w_phi.T
    wct[:, 96:128] = w_g.T
    wog = np.ascontiguousarray((gamma_f * w_o).T)      # [32, 64]
    gwinit = np.zeros((128, NT * 128), dtype=ml_dtypes.bfloat16)
    for t in range(NT):
        gwinit[:, t * 128 + 64:t * 128 + 128] = 1.0

    nc = _get_nc()
    xr = x.reshape(B, C, S)
    # pack wct into the first 128 columns of each sample's x row block
    xcat = np.empty((B, C, 128 + S), dtype=np.float32)
    xcat[:, :, 0:128] = wct[None, :, :]
    xcat[:, :, 128:] = xr
    in_maps = [
        {
            "x": np.ascontiguousarray(xcat[i * BLOC:(i + 1) * BLOC]),
            "wog": wog,
            "gwinit": gwinit,
        }
        for i in range(NCORES)
    ]
    res = run_bass_kernel_spmd(nc, in_maps, core_ids=list(range(NCORES)))
    out = np.concatenate([res.results[i]["out"] for i in range(NCORES)], axis=0)
    return out.reshape(B, C, H, W).astype(np.float32)


if __name__ == "__main__":
    rng = np.random.default_rng(0)
    ins = {
        "x": rng.standard_normal((B, C, H, W), dtype=np.float32),
        "w_theta": (rng.standard_normal((8, 64)) / 8.0).astype(np.float32),
        "w_phi": (rng.standard_normal((8, 64)) / 8.0).astype(np.float32),
        "w_g": (rng.standard_normal((32, 64)) / np.sqrt(64)).astype(np.float32),
        "w_o": (rng.standard_normal((64, 32)) / np.sqrt(32)).astype(np.float32),
        "gamma": np.float32(0.7),
    }
    out = kernel(**ins)
    print("out", out.shape, out.dtype, np.abs(out).mean())


# revision 107
# speedup vs baseline: 1.0075x; 1.0075x over previous
"""Trainium2 Bass kernel for the AttentionBlock problem.

Full inputs:  x [16, 64, 64, 64] f32, w_theta [8, 64], w_phi [8, 64],
              w_g [32, 64], w_o [64, 32], gamma [] (all f32).
Sharding: data-parallel over batch, 2 samples per core on 8 NeuronCores.

Per-sample math (C=64, S=4096, T=S/4=1024):
  theta = w_theta @ x            [8, S]
  phi   = pool2x2(w_phi @ x)     [8, T]
  g     = pool2x2(w_g @ x)       [32, T]
  scoresT[t, s] = sum_c phi[c, t] theta[c, s]
  expT = exp(scoresT)            (no max-subtraction; |scores| <~ 20 is fp32-safe)
  gw[t, m] = sum_c g[c, t] wog[c, m]   (wog = (gamma*w_o)^T, folded on host)
  Fused attn + output conv in ONE accumulation per (chunk, half):
    lhsT = gwo[t-tile] = [gw (64 cols) | ones (64 cols)]  (bf16 [128, 128])
    psum rows 0:64  = oU[m, s] = sum_t gw[t, m] expT[t, s] = gamma*(w_o@attnU)
    psum rows 64:128 = Z[s] broadcast across 64 partitions
  out = oU * recip(Z) + x        (DVE reciprocal + mul + residual add)

This formulation has no separate output conv, no g transposes and no
attnS intermediate; the 2x2 maxpool of phi+g is ONE tensor_reduce
(axis=XY) straight from the conv's PSUM; only theta is staged to SBUF.

Schedule (per core): sample-0 conv/pool/gw phase (wct packed into the
head of x so one DMA delivers both), then the 8 (sample, chunk) bodies
run a two-stage pipeline over 8 t-tile slots:
  body n, slot t: scores+exp(chunk n, tile t); attn of chunk n-1
  interleaved (h0 over slots 0..2, h1 over 3..5) so each [128,512] PSUM
  accumulator frees mid-body, 2 banks suffice, and both norm chains
  clear well before the body ends; recip+mul+add+store of each half is
  emitted right after its accumulation stops. The next sample's
  conv/pool/gw units are spread 3-per-body at slots 3/5/7 so their
  PE/DVE bursts never delay the next body's score tiles.
Each body's t0 scores are pre-emitted at the end of the previous body
(and the last body's t7 scores at slot 6) so no exp ever waits on a
score matmul stuck behind boundary work in the in-order PE queue.
The last chunk additionally interleaves its OWN attn per slot (PSUM from
the then-idle conv pool), splits its final exp in half, and runs one
tail reciprocal on ACT (table load overlaps the DVE chain), so the tail
after the final exp is ~2 matmuls + two overlapped norm chains + store.
ACT (exp, ~67us busy) is the critical engine; PE (~60us) and DVE (~45us)
hide under it. Measured: 80746 ns (baseline 86019 ns).
"""

import sys

if "/opt/trn_rl_repo" not in sys.path:
    sys.path.insert(0, "/opt/trn_rl_repo")

import ml_dtypes
import numpy as np

import concourse.bass as bass
import concourse.tile as tile
from concourse import bacc, mybir
from concourse.bass_utils import run_bass_kernel_spmd

F32 = mybir.dt.float32
F32R = mybir.dt.float32r
BF16 = mybir.dt.bfloat16
AF = mybir.ActivationFunctionType
ALU = mybir.AluOpType

B, C, H, W = 16, 64, 64, 64
S = H * W            # 4096
T = S // 4           # 1024
NCORES = 8
BLOC = B // NCORES   # 2 samples per core
NT = T // 128        # 8 t-tiles
CHUNK = 1024         # s-chunk size
NCH = S // CHUNK     # 4 chunks per sample

_OUT = [None]
_WOGEXT = [None]
_WCTSB = [None]
_COPY1_HOOK = [None]
_HEAD = [None]

# prev-chunk attn schedule: slot t -> (half, [t-tiles]); both halves finish
# by slot 5 so their norms (DVE) clear well before the body ends and the
# last body's tail only carries its own two half-norms
_PREV_SCHED = {1: (0, [0, 1, 2]), 2: (0, [3, 4, 5]), 3: (0, [6, 7]),
               4: (1, [0, 1, 2]), 5: (1, [3, 4, 5]), 6: (1, [6, 7])}
_PREV_NORM = {3: 0, 6: 1}
# fine (last-chunk own attn) schedule: slot t -> [(half, t-tile)]; a tile
# only appears at slot >= tile+1 so PE never waits on ACT, and h0/h1 first
# appear at slots 1/2 (their PSUM comes from the conv pool, idle by then)
_FINE_SCHED = {1: [(0, 0)], 2: [(0, 1), (1, 0)], 3: [(0, 2), (1, 1)],
               4: [(0, 3), (1, 2)], 5: [(0, 4), (1, 3)],
               6: [(0, 5), (1, 4)], 7: [(0, 6), (1, 5), (1, 6)]}
_FINE_POST = [(0, 7), (1, 7)]


def _phase_a(nc, tc, pools, s, x_ext, wog_sb, gwinit_ext):
    """Load x, fused convs, merged maxpools, gw matmul. Returns handles."""
    (pp_sc, pp_at, pp_sm, p_samp, p_chunk) = pools

    # ---- load x: [64, 4096] f32 on partitions 0:64 (base 0 everywhere so
    # SBUF+SBUF tensor ops against it satisfy the equal-base-partition rule)
    # xw_sb: [64, 128 wct | 4096 x] — wct rides in the head of sample 0's
    # x tensor so ONE DMA (one 625ns HWDGE issue slot) delivers both and
    # every later x chunk moves up a slot. x slices live at cols 128+.
    xw_sb = p_samp.tile([64, 128 + S], F32R, tag="x_sb")
    x_sb = xw_sb[:, 128:]
    if s == 0:
        qs = [(0, 640)] + [(640 + q * 512, 640 + (q + 1) * 512)
                           for q in range(7)]
    else:
        qs = [(128, 2176), (2176, 4224)]
    for lo, hi in qs:
        nc.sync.dma_start(xw_sb[:, lo:hi], x_ext[s, :, lo:hi])
    if s == 0:
        # needed only by the gw matmuls ~15us in; keep it out of the
        # latency-critical head of the queue
        nc.sync.dma_start(wog_sb[32:64, :], _WOGEXT[0][:])

    # ---- fused 1x1 convs: [128, 512] psum chunks; conv output rows 0:8
    # theta, 64:72 phi(unpooled), 96:128 g(unpooled): phi+g live in the
    # aligned 64:128 partition window so one op pools both (the DVE/Pool
    # BIR rule forbids a 64-partition access starting at partition 32).
    # Only theta is staged to SBUF (tpg_sb); phi/g are pooled from PSUM.
    tpg_sb = p_samp.tile([8, 4096], F32R, tag="tpg_sb")
    # pooled phi+g: one merged buffer; phi lands on rows 0:8, g on 32:64
    pg_sb = p_samp.tile([64, T], F32R, tag="pg_sb")
    # gwo: per t-tile [128, 128] = [gw 64 cols | ones 64 cols] bf16
    gwo_sb = p_samp.tile([128, NT * 128], BF16, tag="gwo_sb")

    def pool2x2(dst, src):
        # full 2x2 maxpool of one conv chunk in ONE tensor_reduce: src
        # [p, 512] viewed [p, q(h-pair), hh, wo, ww] with the two pooled
        # axes innermost, reduced via axis=XY. Single input, so reading
        # straight from PSUM is legal (one-PSUM-operand rule).
        sv = src.rearrange("p (q hh wo ww) -> p q wo hh ww",
                           q=4, hh=2, wo=32, ww=2)
        dv = dst.rearrange("p (q wo) -> p q wo", wo=32)
        nc.vector.tensor_reduce(dv, sv, mybir.AxisListType.XY, ALU.max)

    # conv chunk k covers h rows 8k..8k+8 == phi/g t-tile k; pool per chunk
    # so downstream scores can start before the whole sample is done.
    # pool2x2 reads the PSUM conv output directly; only theta is staged.
    if s == 0:
        _WCTSB[0] = xw_sb[:, 0:128]

    def conv_chunk(k):
        ps_conv = pp_sm.tile([128, 512], F32, tag="sm", name=f"ps_conv_{s}_{k}")
        nc.tensor.matmul(
            ps_conv[:],
            _WCTSB[0],
            x_sb[:, k * 512:(k + 1) * 512],
            start=True, stop=True,
        )
        # pool phi (rows 64:72) and g (rows 96:128) in one op over rows 64:128
        pool2x2(pg_sb[:, k * 128:(k + 1) * 128],
                ps_conv[64:128, :].bitcast(F32R))
        if s == 0 and k < 2:
            # ACT is idle during startup; the first chunks' theta is the
            # latency-critical piece. copy1 is deferred into the exp-t0
            # split (between the two halves) so exp-t0a doesn't sit behind
            # it in the in-order ACT queue.
            def cp(k=k):
                nc.scalar.copy(tpg_sb[:, k * 512:(k + 1) * 512],
                               ps_conv[0:8, :])
            if k == 1:
                _COPY1_HOOK[0] = cp
            else:
                cp()
        else:
            nc.vector.tensor_copy(tpg_sb[:, k * 512:(k + 1) * 512],
                                  ps_conv[0:8, :])

    def gw_unit():
        # gwinit (ones columns) queued late so it never delays theta/x DMAs.
        nc.sync.dma_start(gwo_sb[:], gwinit_ext[:])
        # gw tiles: gw[t, m] = sum_c g[c, t] wog[c, m]; lhsT = g block
        # [32, 128] (partitions 32:64 of pg_sb), rhs = wog held at
        # partitions 32:64 so operand partition bases match
        for t in range(NT):
            ps_gw = pp_sm.tile([128, 64], F32, tag="sm", name=f"ps_gw_{s}_{t}")
            nc.tensor.matmul(
                ps_gw[:], pg_sb[32:64, t * 128:(t + 1) * 128],
                wog_sb[32:64, :], start=True, stop=True,
            )
            nc.vector.tensor_copy(gwo_sb[:, t * 128:t * 128 + 64], ps_gw[:])

    handles = (x_sb, tpg_sb, pg_sb, gwo_sb)
    units = [lambda k=k: conv_chunk(k) for k in range(8)] + [gw_unit]
    return handles, units


def _act_reciprocal(nc, out, in_):
    """Reciprocal on the ACT engine (table-based, ~1e-3 accurate — fine
    for the 2e-2 gate; bass's wrapper refuses it, so emit the raw
    instruction). Used only in the tail where ACT is idle and the DVE
    norm chain is the critical path."""
    inputs = [nc.scalar.lower_ap(in_)]
    for v in (0.0, 1.0, 0.0):  # bias, scale, alpha immediates
        inputs.append(mybir.ImmediateValue(dtype=mybir.dt.float32, value=v))
    return nc.scalar.add_instruction(
        mybir.InstActivation(
            name=nc.scalar.bass.get_next_instruction_name(),
            func=AF.Reciprocal,
            ins=inputs,
            outs=[nc.scalar.lower_ap(out)],
        )
    )


def _emit_half(nc, pools, s, ch, h, hs, ps_at, act_recip=False,
               dma_engine=None):
    """norm (oU * 1/Z) + residual add + store for one 512-half.
    reciprocal_approx_fast is ~5x cheaper than reciprocal() and accurate
    to ~51 ULP; Z >= 1 always (sum of 1024 positive exps) so its edge
    cases are unreachable. DVE divide doesn't exist at ISA level."""
    (pp_sc, pp_at, pp_sm, p_samp, p_chunk) = pools
    x_sb = hs[0]
    rz_sb = p_chunk.tile([64, 512], F32, tag="rz_sb",
                         name=f"rz_{s}_{ch}_{h}", bufs=2)
    out_sb = p_chunk.tile([64, 512], F32, tag="out_sb",
                          name=f"out_{s}_{ch}_{h}", bufs=3)
    if act_recip:
        _act_reciprocal(nc, rz_sb[:], ps_at[64:128, :])
    else:
        nc.vector.reciprocal(rz_sb[:], ps_at[64:128, :])
    nc.vector.tensor_mul(out_sb[:], ps_at[0:64, :], rz_sb[:])
    s0 = ch * CHUNK + h * 512
    nc.vector.tensor_add(
        out_sb[:], out_sb[:],
        x_sb[:, s0:s0 + 512].bitcast(F32),
    )
    (dma_engine or nc.sync).dma_start(_OUT[0][s, :, s0:s0 + 512], out_sb[:])


def _emit_chunk(nc, pools, s, ch, handles, prev, fine=False, units=(),
                nxt=None):
    """Pipeline body: scores+exp for chunk (s, ch) over 8 t-tile slots,
    with prev chunk's fused attn+oconv matmuls interleaved per
    _PREV_SCHED and (if fine) this chunk's own attn per _FINE_SCHED.
    Deferred phase-A units (next sample's convs/gw) are emitted at mid-
    body slots so they never delay the boundary score tiles. The NEXT
    body's t0 scores are emitted at the end of this body (and the fine
    body's t7 scores at slot 6) so no exp ever waits on a score matmul
    that sits behind boundary work in the in-order PE queue."""
    (pp_sc, pp_at, pp_sm, p_samp, p_chunk) = pools
    units = list(units)

    at_prev = [None, None]
    if prev is not None:
        ps_, ch_, expT_ = prev
        gwo_ = handles[ps_][3]
        for h in range(2):
            at_prev[h] = pp_at.tile([128, 512], F32, tag="at",
                                    name=f"ps_at_{ps_}_{ch_}_{h}")

    x_sb, tpg_sb, pg_sb, gwo_sb = handles[s]
    theta = tpg_sb[:]
    expT = p_chunk.tile([128, NT * CHUNK], BF16, tag="expT",
                        name=f"expT_{s}_{ch}", bufs=3)
    at_cur = [None, None]

    def fine_mm(h, tt):
        if at_cur[h] is None:
            at_cur[h] = pp_sm.tile([128, 512], F32, tag="sm",
                                   name=f"ps_at_{s}_{ch}_{h}")
        nc.tensor.matmul(
            at_cur[h][:],
            gwo_sb[:, tt * 128:(tt + 1) * 128],
            expT[:, tt * CHUNK + h * 512:tt * CHUNK + (h + 1) * 512],
            start=(tt == 0), stop=(tt == NT - 1),
        )

    def sc_mm(ps_sc, t, hh):
        nc.tensor.matmul(
            ps_sc[:, hh * 512:(hh + 1) * 512],
            pg_sb[0:8, t * 128:(t + 1) * 128],
            theta[:, ch * CHUNK + hh * 512:ch * CHUNK + (hh + 1) * 512],
            start=True, stop=True,
        )

    pre_t0, _HEAD[0] = _HEAD[0], None
    pre_t7 = None
    for t in range(NT):
        if t == 0 and pre_t0 is not None:
            ps_sc = pre_t0          # scores already emitted last body
            mms_done = True
        elif t == NT - 1 and pre_t7 is not None:
            ps_sc = pre_t7          # scores already emitted at slot 6
            mms_done = True
        elif s == 0 and ch == 0 and t == 0:
            ps_sc = None            # per-half tiles (see split below)
            mms_done = False
        else:
            ps_sc = pp_sc.tile([128, CHUNK], F32, tag="sc",
                               name=f"ps_sc_{s}_{ch}_{t}")
            mms_done = False
        if (s == 0 and ch == 0 and t == 0) or (fine and t == NT - 1):
            # split boundary tiles in half: the first so ACT starts as soon
            # as conv0's theta half is staged (theta's copy1 is emitted
            # between the halves, keeping exp-t0a ahead of it in the
            # in-order ACT queue; separate per-half tiles also shift the
            # slot rotation a step earlier for the whole stream); the last
            # so the tail's h0 attn+norm chain fires after only the first
            # 512 columns
            for hh in range(2):
                if ps_sc is None:
                    ph = pp_sc.tile([128, 512], F32, tag="sc",
                                    name=f"ps_sc_{s}_{ch}_0{'ab'[hh]}")
                    nc.tensor.matmul(
                        ph[:], pg_sb[0:8, 0:128],
                        theta[:, hh * 512:(hh + 1) * 512],
                        start=True, stop=True,
                    )
                    src = ph[:]
                else:
                    if not mms_done:
                        sc_mm(ps_sc, t, hh)
                    src = ps_sc[:, hh * 512:(hh + 1) * 512]
                nc.scalar.activation(
                    expT[:, t * CHUNK + hh * 512:t * CHUNK + (hh + 1) * 512],
                    src, AF.Exp,
                )
                if hh == 0 and _COPY1_HOOK[0] is not None:
                    _COPY1_HOOK[0]()
                    _COPY1_HOOK[0] = None
        else:
            if not mms_done:
                for hh in range(CHUNK // 512):
                    sc_mm(ps_sc, t, hh)
            nc.scalar.activation(
                expT[:, t * CHUNK:(t + 1) * CHUNK], ps_sc[:], AF.Exp
            )
        if fine and t == NT - 3:
            # pre-emit the final tile's scores (two slots early) so its
            # exps never wait on PE reaching them behind late-body work
            pre_t7 = pp_sc.tile([128, CHUNK], F32, tag="sc",
                                name=f"ps_sc_{s}_{ch}_{NT - 1}")
            sc_mm(pre_t7, NT - 1, 0)
            sc_mm(pre_t7, NT - 1, 1)
        if prev is not None and t in _PREV_SCHED:
            h, tts = _PREV_SCHED[t]
            for tt in tts:
                nc.tensor.matmul(
                    at_prev[h][:],
                    gwo_[:, tt * 128:(tt + 1) * 128],
                    expT_[:, tt * CHUNK + h * 512:tt * CHUNK + (h + 1) * 512],
                    start=(tt == 0), stop=(tt == NT - 1),
                )
            if t in _PREV_NORM:
                _emit_half(nc, pools, ps_, ch_, _PREV_NORM[t],
                           handles[ps_], at_prev[_PREV_NORM[t]])
        if fine:
            for h, tt in _FINE_SCHED.get(t, ()):
                fine_mm(h, tt)
        if t in (3, 5, 7) and units:
            units.pop(0)()

    if fine:
        for h, tt in _FINE_POST:
            fine_mm(h, tt)
        # tail norms: h0's recip on DVE (starts immediately), h1's on ACT
        # (its table load overlaps h0's DVE chain); the final store issues
        # from the idle ACT queue so the two out-DMAs' 625ns HWDGE issue
        # slots run in parallel
        _emit_half(nc, pools, s, ch, 0, handles[s], at_cur[0])
        _emit_half(nc, pools, s, ch, 1, handles[s], at_cur[1],
                   act_recip=True)
        return None
    if nxt is not None:
        # head of the next body: its t0 scores, emitted here so they run
        # ahead of the boundary and its first exp starts back-to-back
        s2, ch2 = nxt
        x2, tpg2, pg2, gwo2 = handles[s2]
        ps_head = pp_sc.tile([128, CHUNK], F32, tag="sc",
                             name=f"ps_sc_{s2}_{ch2}_0")
        for hh in range(2):
            nc.tensor.matmul(
                ps_head[:, hh * 512:(hh + 1) * 512],
                pg2[0:8, 0:128],
                tpg2[:, ch2 * CHUNK + hh * 512:ch2 * CHUNK + (hh + 1) * 512],
                start=True, stop=True,
            )
        _HEAD[0] = ps_head
    return (s, ch, expT)


def build_nc():
    nc = bacc.Bacc("TRN2", target_bir_lowering=False, debug=False,
                   num_devices=NCORES)
    x_ext = nc.dram_tensor("x", [BLOC, C, 128 + S], F32R,
                            kind="ExternalInput").ap()
    wog_ext = nc.dram_tensor("wog", [32, 64], F32R, kind="ExternalInput").ap()
    gwinit_ext = nc.dram_tensor("gwinit", [128, NT * 128], BF16,
                                kind="ExternalInput").ap()
    out_ext = nc.dram_tensor("out", [BLOC, C, S], F32, kind="ExternalOutput").ap()

    with tile.TileContext(nc) as tc:
        with (
            tc.tile_pool(name="wpool", bufs=1) as p_w,
            tc.tile_pool(name="samp", bufs=2) as p_samp,
            tc.tile_pool(name="chunk", bufs=2) as p_chunk,
            tc.tile_pool(name="ppsc", bufs=2, space="PSUM") as pp_sc,
            tc.tile_pool(name="ppat", bufs=2, space="PSUM") as pp_at,
            tc.tile_pool(name="ppsm", bufs=2, space="PSUM") as pp_sm,
        ):
            wog_sb = p_w.tile([64, 64], F32R, tag="wog_sb")
            # dummy exp with no data deps: hoists the ACT exp-table load
            # (1.3us) to t=0, off the conv0 -> copy -> first-exp chain
            dummy_sb = p_w.tile([1, 2], F32, tag="dummy_sb")
            nc.vector.memset(dummy_sb[:], 0)
            nc.scalar.activation(dummy_sb[:], dummy_sb[:], AF.Exp)
            _WOGEXT[0] = wog_ext
            _OUT[0] = out_ext
            pools = (pp_sc, pp_at, pp_sm, p_samp, p_chunk)
            handles = [None] * BLOC
            handles[0], units0 = _phase_a(nc, tc, pools, 0, x_ext,
                                          wog_sb, gwinit_ext)
            # sample 0's convs run inline (they feed body (0,0) directly);
            # its gw unit is deferred into body (0,0)'s mid-slots, giving
            # the otherwise attn-less first body PE work that keeps the
            # array's p-state hot for the boundary head matmuls
            for u in units0[:-1]:
                u()
            prev = None
            pending = [units0[-1]]  # deferred phase-A work units
            seq = [(s, ch) for s in range(BLOC) for ch in range(NCH)]
            for i, (s, ch) in enumerate(seq):
                last = i == len(seq) - 1
                nxt = None if last else seq[i + 1]
                prev = _emit_chunk(nc, pools, s, ch, handles, prev,
                                   fine=last, units=pending[:3], nxt=nxt)
                pending = pending[3:]
                if ch == 0 and s + 1 < BLOC:
                    handles[s + 1], pending = _phase_a(
                        nc, tc, pools, s + 1, x_ext, wog_sb,
                        gwinit_ext)

    nc.compile()
    return nc


_NC_CACHE = None


def _get_nc():
    global _NC_CACHE
    if _NC_CACHE is None:
        _NC_CACHE = build_nc()
    return _NC_CACHE


def kernel(x, w_theta, w_phi, w_g, w_o, gamma):
    x = np.ascontiguousarray(np.asarray(x, dtype=np.float32))
    w_theta = np.asarray(w_theta, dtype=np.float32)
    w_phi = np.asarray(w_phi, dtype=np.float32)
    w_g = np.asarray(w_g, dtype=np.float32)
    w_o = np.asarray(w_o, dtype=np.float32)
    gamma_f = float(np.asarray(gamma, dtype=np.float32))

    # lhsT for the fused conv: [64, 128] = [w_theta.T | pad | w_phi.T | pad |
    # w_g.T] (phi at col 64, g at col 96 so phi+g land in the aligned 64:128
    # partition window of the conv output).
    wct = np.zeros((64, 128), dtype=np.float32)
    wct[:, 0:8] = w_theta.T
    wct[:, 64:72] = w_phi.T
    wct[:, 96:128] = w_g.T
    wog = np.ascontiguousarray((gamma_f * w_o).T)      # [32, 64]
    gwinit = np.zeros((128, NT * 128), dtype=ml_dtypes.bfloat16)
    for t in range(NT):
        gwinit[:, t * 128 + 64:t * 128 + 128] = 1.0

    nc = _get_nc()
    xr = x.reshape(B, C, S)
    # pack wct into the first 128 columns of each sample's x row block
    xcat = np.empty((B, C, 128 + S), dtype=np.float32)
    xcat[:, :, 0:128] = wct[None, :, :]
    xcat[:, :, 128:] = xr
    in_maps = [
        {
            "x": np.ascontiguousarray(xcat[i * BLOC:(i + 1) * BLOC]),
            "wog": wog,
            "gwinit": gwinit,
        }
        for i in range(NCORES)
    ]
    res = run_bass_kernel_spmd(nc, in_maps, core_ids=list(range(NCORES)))
    out = np.concatenate([res.results[i]["out"] for i in range(NCORES)], axis=0)
    return out.reshape(B, C, H, W).astype(np.float32)


if __name__ == "__main__":
    rng = np.random.default_rng(0)
    ins = {
        "x": rng.standard_normal((B, C, H, W), dtype=np.float32),
        "w_theta": (rng.standard_normal((8, 64)) / 8.0).astype(np.float32),
        "w_phi": (rng.standard_normal((8, 64)) / 8.0).astype(np.float32),
        "w_g": (rng.standard_normal((32, 64)) / np.sqrt(64)).astype(np.float32),
        "w_o": (rng.standard_normal((64, 32)) / np.sqrt(32)).astype(np.float32),
        "gamma": np.float32(0.7),
    }
    out = kernel(**ins)
    print("out", out.shape, out.dtype, np.abs(out).mean())
